# revision 5
# baseline (speedup 1.0000x reference)
"""ChebyKAN linear layer on 8 Trainium2 NeuronCores.

Math: y[b,o] = sum_{i,d} T_d(w[b,i]) * C[i,o,d], with w = tanh(tanh(x)) and
T_d the Chebyshev polynomials (cos(d*arccos(w)) == T_d(w) for |w|<=1).

The ACT engine has no arccos/cos, so the device evaluates the Chebyshev-product
basis phi = [T1, T1^2, T1*T2, T2^2, T2*T3, T3^2, T3*T4, T4^2] built from
Square/multiply ops (T2, T4 and the T3 helper come from cheap affine ops). Via
T_{2k} = 2*T_k^2-1 and T_{m+n} = 2*T_m*T_n - T_{m-n}, an exact host-side
linear transform maps Chebyshev coefficients onto this basis with O(1)
conditioning; the constant column folds into a per-o bias added during PSUM
evacuation.

Sharding: data-parallel over batch b (16384 -> 2048/core); coeffs replicated.
x is pre-transposed on the host so the contraction dim (c_in) lands on SBUF
partitions; the kernel computes y^T per core and the host transposes back.

Matmul operands are bf16 (1 cycle/row streaming -- measured 213ns/matmul
start-to-start at 512 rows, the PE floor -- and half the HBM/SBUF traffic of
f32r for the weight stream). The basis chain stays full fp32; each matmul
operand is rounded to bf16 exactly once; PSUM accumulates fp32. Measured
end-to-end error ~3e-3 vs the 2e-2 gate.

Schedule (from NTFF profiles): per-core time = prologue + 109.2us of PE
streaming + drain + ~7.5us fixed NRT teardown, so the tuning targets are the
edges. The batch is processed in six PSUM phases sized 256/256/512/512/256/256:
narrow head phases cut the first-matmul latency (a 128KB first x-sliver lands
~2.5us sooner than 256KB, and the tanh chain on half tiles is ~2x shorter);
narrow tail phases shrink the final evacuation + output DMA. Elementwise work
is spread ACT(6)/DVE(7)/Pool(3) per row-block so no engine exceeds ~75% of the
6.8us block budget (ACT at 8 ops was 92% busy and stalled the PE). tile_wait_
until stamps per row-block keep each in-order engine queue in consumption order
(the scheduler otherwise predicts DMA completions optimistically and
head-of-line blocks the ACT queue on a late transfer). DMA: ~620ns sequencer
issue + ~1.5us doorbell-to-packet wake per transfer; a tiny bias load goes
first on the gpsimd ring to absorb the wake. Weights load as one 1MB bf16 DMA
per 128-row block (block 0 split j-wise on gpsimd in PE consumption order).
Output DMAs ride the gpsimd SWDGE ring (slow ~110GB/s but fully hidden)
except the last phase's, which use the two fast HWDGE rings (sync/scalar,
~360GB/s) to drain in ~1.5us. A short fp32 dummy-matmul burst warms the PE
HAM clock gate (1.2->2.4GHz) during the DMA ramp.
"""

import sys

if "/opt/trn_rl_repo" not in sys.path:
    sys.path.append("/opt/trn_rl_repo")

import numpy as np
import ml_dtypes

import concourse.bacc as bacc
import concourse.tile as tile
from concourse import mybir
from concourse.bass_utils import run_bass_kernel_spmd

DEGREE = 8
B, C_IN, C_OUT = 16384, 512, 512
N_CORES = 8
NB = B // N_CORES            # 2048 batch rows per core
N_IB = C_IN // 128           # 4 contraction row-blocks
N_J = DEGREE                 # basis funcs phi_1..phi_8 (constant -> bias)
F32 = mybir.dt.float32
BF16 = mybir.dt.bfloat16

# (batch offset, width) per PSUM accumulation phase: narrow head phases for a
# fast pipeline ramp-in, narrow tail phases for a fast drain.
PHASES = [(0, 256), (256, 256), (512, 512), (1024, 512), (1536, 256), (1792, 256)]
assert sum(w for _, w in PHASES) == NB

_CACHE = {}


def _build():
    nc = bacc.Bacc("TRN2", target_bir_lowering=False, debug=False)
    xt = nc.dram_tensor("xt", [C_IN, NB], F32, kind="ExternalInput")
    wmat = nc.dram_tensor("wmat", [C_IN, N_J * C_OUT], BF16, kind="ExternalInput")
    biasv = nc.dram_tensor("biasv", [128, 4], F32, kind="ExternalInput")
    yt = nc.dram_tensor("yt", [C_OUT, NB], F32, kind="ExternalOutput")

    Tanh = mybir.ActivationFunctionType.Tanh
    Square = mybir.ActivationFunctionType.Square
    Identity = mybir.ActivationFunctionType.Identity
    ALU_MULT = mybir.AluOpType.mult
    ALU_ADD = mybir.AluOpType.add

    with tile.TileContext(nc) as tc:
        with (
            tc.tile_pool(name="const", bufs=1) as const_pool,
            tc.tile_pool(name="wts", bufs=1) as wpool,
            tc.tile_pool(name="pows", bufs=2) as ppool,
            tc.tile_pool(name="outs", bufs=2) as opool,
            tc.tile_pool(name="psum", bufs=2, space="PSUM") as pspool,
        ):
            # PE warm-up fodder: the HAM clock gate keeps the PE at 1.2GHz
            # until ~3.4us of sustained activity; fp32 dummy matmuls (two
            # LOW/HIGH passes each, ~420ns apiece) warm it up while the
            # first DMAs are in flight. The memset rides gpsimd, whose
            # framework preamble retires earliest.
            dummy = const_pool.tile([128, 128], F32, tag="dummy")
            nc.gpsimd.memset(dummy[:], 0.0)
            cm1 = const_pool.tile([128, 1], F32, tag="cm1")
            nc.gpsimd.memset(cm1[:], -1.0)
            dps = pspool.tile([128, 512], F32, tag="ps3", name="dps")
            for _ in range(8):
                nc.tensor.matmul(
                    dps[:, 0:128], lhsT=dummy[:], rhs=dummy[:],
                    start=True, stop=True,
                )

            # Tiny bias load first on gpsimd: absorbs the ~1.5us DMA-engine
            # wake so the critical x sliver starts moving sooner.
            bias_t = const_pool.tile([128, 4], F32)
            nc.gpsimd.dma_start(out=bias_t[:], in_=biasv.ap())

            # Weights: one [128, 8*512] bf16 tile per contraction row-block.
            # Row-block 0 goes j-chunk-wise on the gpsimd (SWDGE) ring in PE
            # consumption order, concurrent with the sync-ring x slivers;
            # row-blocks 1-3 are single 1MB transfers on the sync ring.
            w_sb = []
            wt0 = wpool.tile([128, N_J * C_OUT], BF16, tag="wc0", name="wc0")
            for j in range(N_J):
                nc.gpsimd.dma_start(
                    out=wt0[:, j * C_OUT : (j + 1) * C_OUT],
                    in_=wmat.ap()[0:128, j * C_OUT : (j + 1) * C_OUT],
                )
            w_sb.append(wt0)

            # Phase-0 x slivers first on the sync ring: the ib=0 sliver
            # (128KB) gates the tanh chain and therefore the first matmul.
            xlbs = []
            xlb0 = ppool.tile([128, N_IB, 256], F32, tag="xlb0", bufs=1)
            for ib in range(N_IB):
                nc.sync.dma_start(
                    out=xlb0[:, ib, :],
                    in_=xt.ap()[ib * 128 : (ib + 1) * 128, 0:256],
                )
            xlbs.append(xlb0)

            def load_xlb(pi):
                off, wd = PHASES[pi]
                xlb = ppool.tile(
                    [128, N_IB, wd], F32, tag=f"xlb{pi}", bufs=1, name=f"xlb{pi}"
                )
                nc.sync.dma_start(
                    out=xlb[:],
                    in_=xt.ap()[:, off : off + wd].rearrange(
                        "(ib p) b -> p ib b", p=128
                    ),
                )
                xlbs.append(xlb)

            load_xlb(1)
            for ib in range(1, N_IB):
                wt = wpool.tile([128, N_J * C_OUT], BF16, tag=f"wc{ib}", name=f"wc{ib}")
                nc.sync.dma_start(
                    out=wt[:],
                    in_=wmat.ap()[ib * 128 : (ib + 1) * 128, :],
                )
                w_sb.append(wt)
            for pi in range(2, len(PHASES)):
                load_xlb(pi)

            def w_chunk(ib, j, oc):
                return w_sb[ib][:, j * C_OUT + oc * 128 : j * C_OUT + (oc + 1) * 128]

            # Virtual-time stamps (ms) keep each in-order engine queue in
            # consumption order; the value tracks when each row-block's
            # matmuls can actually start (10.5us prologue + PE time so far).
            t_ms = 0.0105
            for pi, (off, wd) in enumerate(PHASES):
                ps = [
                    pspool.tile([128, 512], F32, tag=f"ps{oc}", name=f"ps{oc}_{pi}")
                    for oc in range(4)
                ]
                xlb = xlbs[pi]
                for ib in range(N_IB):
                    with tc.tile_wait_until(t_ms):
                        # Chebyshev-product basis, full-fp32 chain; the
                        # critical path (t1 -> f2 -> t2 -> f4 -> t4) stays on
                        # ACT, DVE/Pool produce the bf16 matmul operands.
                        nc.scalar.activation(xlb[:, ib, :], xlb[:, ib, :], Tanh)
                        t1 = ppool.tile([128, 512], F32, tag="t1", bufs=3)
                        nc.scalar.activation(t1[:, :wd], xlb[:, ib, :], Tanh)
                        f2 = ppool.tile([128, 512], F32, tag="f2", bufs=3)
                        t2 = ppool.tile([128, 512], F32, tag="t2", bufs=3)
                        u3 = ppool.tile([128, 512], F32, tag="u3", bufs=3)
                        t3 = ppool.tile([128, 512], F32, tag="t3", bufs=3)
                        f4 = ppool.tile([128, 512], F32, tag="f4", bufs=3)
                        t4 = ppool.tile([128, 512], F32, tag="t4", bufs=3)
                        t1r = ppool.tile([128, 512], BF16, tag="t1r", bufs=3)
                        f2r = ppool.tile([128, 512], BF16, tag="f2r", bufs=3)
                        f3 = ppool.tile([128, 512], BF16, tag="f3", bufs=3)
                        f4r = ppool.tile([128, 512], BF16, tag="f4r", bufs=3)
                        f5 = ppool.tile([128, 512], BF16, tag="f5", bufs=3)
                        f6 = ppool.tile([128, 512], BF16, tag="f6", bufs=3)
                        f7 = ppool.tile([128, 512], BF16, tag="f7", bufs=3)
                        f8 = ppool.tile([128, 512], BF16, tag="f8", bufs=3)
                        s = slice(0, wd)
                        nc.vector.tensor_copy(t1r[:, s], t1[:, s])
                        nc.scalar.activation(f2[:, s], t1[:, s], Square)
                        nc.gpsimd.tensor_copy(f2r[:, s], f2[:, s])
                        nc.scalar.activation(
                            t2[:, s], f2[:, s], Identity, bias=cm1[:], scale=2.0
                        )
                        nc.vector.tensor_scalar(
                            u3[:, s], f2[:, s], 4.0, -3.0, ALU_MULT, ALU_ADD
                        )
                        nc.vector.tensor_mul(t3[:, s], t1[:, s], u3[:, s])
                        nc.vector.tensor_mul(f3[:, s], t1[:, s], t2[:, s])
                        nc.scalar.activation(f4[:, s], t2[:, s], Square)
                        nc.gpsimd.tensor_copy(f4r[:, s], f4[:, s])
                        nc.scalar.activation(
                            t4[:, s], f4[:, s], Identity, bias=cm1[:], scale=2.0
                        )
                        nc.vector.tensor_mul(f5[:, s], t2[:, s], t3[:, s])
                        nc.vector.tensor_mul(f6[:, s], t3[:, s], t3[:, s])
                        nc.gpsimd.tensor_mul(f7[:, s], t3[:, s], t4[:, s])
                        nc.vector.tensor_mul(f8[:, s], t4[:, s], t4[:, s])
                        chunks = [t1r, f2r, f3, f4r, f5, f6, f7, f8]
                        if ib < N_IB - 1:
                            order = [(j, oc) for j in range(N_J) for oc in range(4)]
                        else:
                            # oc-major on the last row-block: accumulation
                            # groups finish staggered -> evacuation overlaps
                            # the matmul stream
                            order = [(j, oc) for oc in range(4) for j in range(N_J)]
                        for j, oc in order:
                            nc.tensor.matmul(
                                ps[oc][:, s],
                                lhsT=w_chunk(ib, j, oc),
                                rhs=chunks[j][:, s],
                                start=(ib == 0 and j == 0),
                                stop=(ib == N_IB - 1 and j == N_J - 1),
                            )
                            if ib == N_IB - 1 and j == N_J - 1:
                                osb = opool.tile(
                                    [128, 512], F32, tag=f"osb{oc}", name=f"osb{oc}"
                                )
                                nc.scalar.activation(
                                    osb[:, s], ps[oc][:, s], Identity,
                                    bias=bias_t[:, oc : oc + 1],
                                )
                                if pi == len(PHASES) - 1:
                                    # final outputs on the fast HWDGE rings
                                    out_eng = (
                                        nc.scalar, nc.sync, nc.scalar, nc.sync
                                    )[oc]
                                else:
                                    out_eng = nc.gpsimd
                                out_eng.dma_start(
                                    out=yt.ap()[oc * 128 : (oc + 1) * 128, off : off + wd],
                                    in_=osb[:, s],
                                )
                    t_ms += wd * 128 * 0.4167 * 1e-6
    nc.compile()
    return nc


def _host_transform(cheby_coeffs):
    # Map Chebyshev coefficients onto the device phi basis:
    # phi = [T1, T1^2, T1*T2, T2^2, T2*T3, T3^2, T3*T4, T4^2] and a constant.
    # T_{2k} = 2*T_k^2 - 1, T_{m+n} = 2*T_m*T_n - T_{m-n} =>
    #   y = bias + (C1-C3-C5-C7)*T1 + sum_{d=2..8} 2*C_d * phi_{d-1}
    #   bias_o = sum_i (C0 - C2 - C4 - C6 - C8)
    C64 = cheby_coeffs.astype(np.float64)
    bias = (C64[..., 0] - C64[..., 2] - C64[..., 4] - C64[..., 6] - C64[..., 8]).sum(
        axis=0
    )
    W = np.empty((C_IN, C_OUT, N_J), np.float64)
    W[..., 0] = C64[..., 1] - C64[..., 3] - C64[..., 5] - C64[..., 7]
    for d in range(2, DEGREE + 1):
        W[..., d - 1] = 2.0 * C64[..., d]
    # [i, j*512+o]: per-partition-contiguous coefficient rows, bf16 on device
    Wd = np.ascontiguousarray(
        W.transpose(0, 2, 1).reshape(C_IN, N_J * C_OUT).astype(ml_dtypes.bfloat16)
    )
    bias_dev = np.ascontiguousarray(bias.reshape(4, 128).T.astype(np.float32))
    return Wd, bias_dev


def kernel(x, cheby_coeffs):
    x = np.asarray(x, dtype=np.float32)
    cheby_coeffs = np.asarray(cheby_coeffs, dtype=np.float32)
    if "nc" not in _CACHE:
        _CACHE["nc"] = _build()
    nc = _CACHE["nc"]

    Wd, bias_dev = _host_transform(cheby_coeffs)
    xT = np.ascontiguousarray(x.T)                       # [c_in, b]
    in_maps = [
        {
            "xt": np.ascontiguousarray(xT[:, c * NB : (c + 1) * NB]),
            "wmat": Wd,
            "biasv": bias_dev,
        }
        for c in range(N_CORES)
    ]
    res = run_bass_kernel_spmd(nc, in_maps, core_ids=list(range(N_CORES)))
    y = np.concatenate([res.results[c]["yt"].T for c in range(N_CORES)], axis=0)
    return y


# revision 6
# speedup vs baseline: 1.2092x; 1.2092x over previous
"""ChebyKAN linear layer on 8 Trainium2 NeuronCores.

Math: y[b,o] = sum_{i,d} T_d(w[b,i]) * C[i,o,d], with w = tanh(tanh(x)) and
T_d the Chebyshev polynomials (cos(d*arccos(w)) == T_d(w) for |w|<=1).

The ACT engine has no arccos/cos, so the device evaluates the Chebyshev-product
basis phi = [T1, T1^2, T1*T2, T2^2, T2*T3, T3^2, T3*T4, T4^2] built from
Square/multiply ops (T2, T4 and the T3 helper come from cheap affine ops). Via
T_{2k} = 2*T_k^2-1 and T_{m+n} = 2*T_m*T_n - T_{m-n}, an exact host-side
linear transform maps Chebyshev coefficients onto this basis with O(1)
conditioning; the constant column folds into a per-o bias added during PSUM
evacuation.

Sharding: data-parallel over batch b (16384 -> 2048/core); coeffs replicated.
x is pre-transposed on the host so the contraction dim (c_in) lands on SBUF
partitions; the kernel computes y^T per core and the host transposes back.

Everything on device is fp16: x ships as fp16 (2MB/core), weights as fp16
scaled x16 on the host (4MB/core; the raw values ~1e-4 would graze the fp16
subnormal floor, and the 1/16 rescale folds into the evacuation's activation
scale for free), and the whole basis chain computes in fp16 (10 mantissa
bits -> measured end-to-end error ~1.2e-3 vs the 2e-2 gate, better than a
single-rounded bf16 scheme). fp16 matmuls stream 1 cycle/row -- measured
213ns per 512-row matmul, the PE floor -- PSUM accumulates fp32, and the
fp16-native chain needs no separate cast ops: ACT runs the critical path
(tanh -> tanh -> Square/affine, 6 ops/row-block) writing fp16 directly, DVE
runs the 7 off-critical multiplies at 16-bit double rate. An 8-op ACT chain
ran at 92% occupancy and intermittently starved the PE, which then bounced
the HAM clock gate between 1.2/2.4GHz; gpsimd (Q7) elementwise measured
~1.5us per [128,512] cast, 2-3x the cost model, so it gets no chain work.

Schedule (from NTFF profiles): per-core time = prologue + 109.2us of PE
streaming + drain + ~7.5us fixed NRT teardown, so the tuning targets are the
edges. The batch runs in six PSUM phases sized 256/256/512/512/256/256:
narrow head phases cut first-matmul latency (the 64KB first x-sliver lands
~2.5us sooner, and the half-width tanh chain is ~2x shorter); narrow tail
phases shrink the final evacuation + output DMA. tile_wait_until stamps per
row-block keep each in-order engine queue in consumption order (the
scheduler otherwise predicts DMA completions optimistically and head-of-line
blocks the ACT queue on a late transfer). DMA: ~620ns sequencer issue +
~1.5us doorbell-to-packet wake per transfer; a tiny bias load goes first on
the gpsimd ring to absorb the wake. Weights load as one 1MB fp16 DMA per
128-row contraction block (block 0 split j-wise on gpsimd in PE consumption
order, racing the sync-ring x slivers). Output DMAs ride the gpsimd SWDGE
ring (slow ~110GB/s but fully hidden) except the last phase's, which use the
two fast HWDGE rings (sync/scalar, ~360GB/s) to drain in ~1.5us. A short
fp32 dummy-matmul burst warms the PE HAM clock gate (1.2->2.4GHz) during the
DMA ramp.
"""

import sys

if "/opt/trn_rl_repo" not in sys.path:
    sys.path.append("/opt/trn_rl_repo")

import numpy as np

import concourse.bacc as bacc
import concourse.tile as tile
from concourse import mybir
from concourse.bass_utils import run_bass_kernel_spmd

DEGREE = 8
B, C_IN, C_OUT = 16384, 512, 512
N_CORES = 8
NB = B // N_CORES            # 2048 batch rows per core
N_IB = C_IN // 128           # 4 contraction row-blocks
N_J = DEGREE                 # basis funcs phi_1..phi_8 (constant -> bias)
F32 = mybir.dt.float32
F16 = mybir.dt.float16
W_SCALE = 16.0               # host premultiplies weights; evac scales by 1/16

# (batch offset, width) per PSUM accumulation phase: narrow head phases for a
# fast pipeline ramp-in, narrow tail phases for a fast drain.
PHASES = [(0, 256), (256, 256), (512, 512), (1024, 512), (1536, 256), (1792, 256)]
assert sum(w for _, w in PHASES) == NB

_CACHE = {}


def _build():
    nc = bacc.Bacc("TRN2", target_bir_lowering=False, debug=False)
    xt = nc.dram_tensor("xt", [C_IN, NB], F16, kind="ExternalInput")
    wmat = nc.dram_tensor("wmat", [C_IN, N_J * C_OUT], F16, kind="ExternalInput")
    biasv = nc.dram_tensor("biasv", [128, 4], F32, kind="ExternalInput")
    yt = nc.dram_tensor("yt", [C_OUT, NB], F32, kind="ExternalOutput")

    Tanh = mybir.ActivationFunctionType.Tanh
    Square = mybir.ActivationFunctionType.Square
    Identity = mybir.ActivationFunctionType.Identity
    ALU_MULT = mybir.AluOpType.mult
    ALU_ADD = mybir.AluOpType.add

    with tile.TileContext(nc) as tc:
        with (
            tc.tile_pool(name="const", bufs=1) as const_pool,
            tc.tile_pool(name="wts", bufs=1) as wpool,
            tc.tile_pool(name="pows", bufs=2) as ppool,
            tc.tile_pool(name="outs", bufs=2) as opool,
            tc.tile_pool(name="psum", bufs=2, space="PSUM") as pspool,
        ):
            # PE warm-up fodder: the HAM clock gate keeps the PE at 1.2GHz
            # until ~3.4us of sustained activity; fp32 dummy matmuls (two
            # LOW/HIGH passes each, ~420ns apiece) warm it up while the
            # first DMAs are in flight. The memset rides gpsimd, whose
            # framework preamble retires earliest.
            dummy = const_pool.tile([128, 128], F32, tag="dummy")
            nc.gpsimd.memset(dummy[:], 0.0)
            cm1 = const_pool.tile([128, 1], F16, tag="cm1")
            nc.gpsimd.memset(cm1[:], -1.0)
            dps = pspool.tile([128, 512], F32, tag="ps3", name="dps")
            for _ in range(8):
                nc.tensor.matmul(
                    dps[:, 0:128], lhsT=dummy[:], rhs=dummy[:],
                    start=True, stop=True,
                )

            # Tiny bias load first on gpsimd: absorbs the ~1.5us DMA-engine
            # wake so the critical x sliver starts moving sooner.
            bias_t = const_pool.tile([128, 4], F32)
            nc.gpsimd.dma_start(out=bias_t[:], in_=biasv.ap())

            # Weights: one [128, 8*512] fp16 tile per contraction row-block.
            # Row-block 0 goes j-chunk-wise on the gpsimd (SWDGE) ring in PE
            # consumption order, concurrent with the sync-ring x slivers;
            # row-blocks 1-3 are single 1MB transfers on the sync ring.
            w_sb = []
            wt0 = wpool.tile([128, N_J * C_OUT], F16, tag="wc0", name="wc0")
            for j in range(N_J):
                nc.gpsimd.dma_start(
                    out=wt0[:, j * C_OUT : (j + 1) * C_OUT],
                    in_=wmat.ap()[0:128, j * C_OUT : (j + 1) * C_OUT],
                )
            w_sb.append(wt0)

            # Phase-0 x slivers first on the sync ring: the ib=0 sliver
            # (64KB) gates the tanh chain and therefore the first matmul.
            xlbs = []
            xlb0 = ppool.tile([128, N_IB, 256], F16, tag="xlb0", bufs=1)
            for ib in range(N_IB):
                nc.sync.dma_start(
                    out=xlb0[:, ib, :],
                    in_=xt.ap()[ib * 128 : (ib + 1) * 128, 0:256],
                )
            xlbs.append(xlb0)

            def load_xlb(pi):
                off, wd = PHASES[pi]
                xlb = ppool.tile(
                    [128, N_IB, wd], F16, tag=f"xlb{pi}", bufs=1, name=f"xlb{pi}"
                )
                nc.sync.dma_start(
                    out=xlb[:],
                    in_=xt.ap()[:, off : off + wd].rearrange(
                        "(ib p) b -> p ib b", p=128
                    ),
                )
                xlbs.append(xlb)

            load_xlb(1)
            for ib in range(1, N_IB):
                wt = wpool.tile([128, N_J * C_OUT], F16, tag=f"wc{ib}", name=f"wc{ib}")
                nc.sync.dma_start(
                    out=wt[:],
                    in_=wmat.ap()[ib * 128 : (ib + 1) * 128, :],
                )
                w_sb.append(wt)
            for pi in range(2, len(PHASES)):
                load_xlb(pi)

            def w_chunk(ib, j, oc):
                return w_sb[ib][:, j * C_OUT + oc * 128 : j * C_OUT + (oc + 1) * 128]

            # Virtual-time stamps (ms) keep each in-order engine queue in
            # consumption order; the value tracks when each row-block's
            # matmuls can actually start (10.5us prologue + PE time so far).
            t_ms = 0.0105
            for pi, (off, wd) in enumerate(PHASES):
                ps = [
                    pspool.tile([128, 512], F32, tag=f"ps{oc}", name=f"ps{oc}_{pi}")
                    for oc in range(4)
                ]
                xlb = xlbs[pi]
                for ib in range(N_IB):
                    with tc.tile_wait_until(t_ms):
                        # fp16 Chebyshev-product basis chain. Critical path
                        # (tanh -> tanh -> Square/affine) stays on ACT; DVE
                        # runs the off-critical multiplies at 16-bit 2x rate.
                        s = slice(0, wd)
                        t1 = ppool.tile([128, 512], F16, tag="t1", bufs=3)
                        f2 = ppool.tile([128, 512], F16, tag="f2", bufs=3)
                        t2 = ppool.tile([128, 512], F16, tag="t2", bufs=3)
                        u3 = ppool.tile([128, 512], F16, tag="u3", bufs=3)
                        t3 = ppool.tile([128, 512], F16, tag="t3", bufs=3)
                        f3 = ppool.tile([128, 512], F16, tag="f3", bufs=3)
                        f4 = ppool.tile([128, 512], F16, tag="f4", bufs=3)
                        t4 = ppool.tile([128, 512], F16, tag="t4", bufs=3)
                        f5 = ppool.tile([128, 512], F16, tag="f5", bufs=3)
                        f6 = ppool.tile([128, 512], F16, tag="f6", bufs=3)
                        f7 = ppool.tile([128, 512], F16, tag="f7", bufs=3)
                        f8 = ppool.tile([128, 512], F16, tag="f8", bufs=3)
                        nc.scalar.activation(xlb[:, ib, :], xlb[:, ib, :], Tanh)
                        nc.scalar.activation(t1[:, s], xlb[:, ib, :], Tanh)
                        nc.scalar.activation(f2[:, s], t1[:, s], Square)
                        nc.scalar.activation(
                            t2[:, s], f2[:, s], Identity, bias=cm1[:], scale=2.0
                        )
                        nc.vector.tensor_scalar(
                            u3[:, s], f2[:, s], 4.0, -3.0, ALU_MULT, ALU_ADD
                        )
                        nc.vector.tensor_mul(t3[:, s], t1[:, s], u3[:, s])
                        nc.vector.tensor_mul(f3[:, s], t1[:, s], t2[:, s])
                        nc.scalar.activation(f4[:, s], t2[:, s], Square)
                        nc.scalar.activation(
                            t4[:, s], f4[:, s], Identity, bias=cm1[:], scale=2.0
                        )
                        nc.vector.tensor_mul(f5[:, s], t2[:, s], t3[:, s])
                        nc.vector.tensor_mul(f6[:, s], t3[:, s], t3[:, s])
                        nc.vector.tensor_mul(f7[:, s], t3[:, s], t4[:, s])
                        nc.vector.tensor_mul(f8[:, s], t4[:, s], t4[:, s])
                        chunks = [t1, f2, f3, f4, f5, f6, f7, f8]
                        if ib < N_IB - 1:
                            order = [(j, oc) for j in range(N_J) for oc in range(4)]
                        else:
                            # oc-major on the last row-block: accumulation
                            # groups finish staggered -> evacuation overlaps
                            # the matmul stream
                            order = [(j, oc) for oc in range(4) for j in range(N_J)]
                        for j, oc in order:
                            nc.tensor.matmul(
                                ps[oc][:, s],
                                lhsT=w_chunk(ib, j, oc),
                                rhs=chunks[j][:, s],
                                start=(ib == 0 and j == 0),
                                stop=(ib == N_IB - 1 and j == N_J - 1),
                            )
                            if ib == N_IB - 1 and j == N_J - 1:
                                osb = opool.tile(
                                    [128, 512], F32, tag=f"osb{oc}", name=f"osb{oc}"
                                )
                                nc.scalar.activation(
                                    osb[:, s], ps[oc][:, s], Identity,
                                    bias=bias_t[:, oc : oc + 1],
                                    scale=1.0 / W_SCALE,
                                )
                                if pi == len(PHASES) - 1:
                                    # final outputs on the fast HWDGE rings
                                    out_eng = (
                                        nc.scalar, nc.sync, nc.scalar, nc.sync
                                    )[oc]
                                else:
                                    out_eng = nc.gpsimd
                                out_eng.dma_start(
                                    out=yt.ap()[oc * 128 : (oc + 1) * 128, off : off + wd],
                                    in_=osb[:, s],
                                )
                    t_ms += wd * 128 * 0.4167 * 1e-6
    nc.compile()
    return nc


def _host_transform(cheby_coeffs):
    # Map Chebyshev coefficients onto the device phi basis:
    # phi = [T1, T1^2, T1*T2, T2^2, T2*T3, T3^2, T3*T4, T4^2] and a constant.
    # T_{2k} = 2*T_k^2 - 1, T_{m+n} = 2*T_m*T_n - T_{m-n} =>
    #   y = bias + (C1-C3-C5-C7)*T1 + sum_{d=2..8} 2*C_d * phi_{d-1}
    #   bias_o = sum_i (C0 - C2 - C4 - C6 - C8)
    C64 = cheby_coeffs.astype(np.float64)
    bias = (C64[..., 0] - C64[..., 2] - C64[..., 4] - C64[..., 6] - C64[..., 8]).sum(
        axis=0
    )
    W = np.empty((C_IN, C_OUT, N_J), np.float64)
    W[..., 0] = C64[..., 1] - C64[..., 3] - C64[..., 5] - C64[..., 7]
    for d in range(2, DEGREE + 1):
        W[..., d - 1] = 2.0 * C64[..., d]
    # [i, j*512+o]: per-partition-contiguous coefficient rows; fp16 on device,
    # premultiplied by W_SCALE (undone by the evacuation's activation scale)
    # to clear the fp16 subnormal floor.
    Wd = np.ascontiguousarray(
        (W.transpose(0, 2, 1).reshape(C_IN, N_J * C_OUT) * W_SCALE).astype(np.float16)
    )
    bias_dev = np.ascontiguousarray(bias.reshape(4, 128).T.astype(np.float32))
    return Wd, bias_dev


def kernel(x, cheby_coeffs):
    x = np.asarray(x, dtype=np.float32)
    cheby_coeffs = np.asarray(cheby_coeffs, dtype=np.float32)
    if "nc" not in _CACHE:
        _CACHE["nc"] = _build()
    nc = _CACHE["nc"]

    Wd, bias_dev = _host_transform(cheby_coeffs)
    xT = np.ascontiguousarray(x.T)                       # [c_in, b]
    in_maps = [
        {
            "xt": np.ascontiguousarray(xT[:, c * NB : (c + 1) * NB].astype(np.float16)),
            "wmat": Wd,
            "biasv": bias_dev,
        }
        for c in range(N_CORES)
    ]
    res = run_bass_kernel_spmd(nc, in_maps, core_ids=list(range(N_CORES)))
    y = np.concatenate([res.results[c]["yt"].T for c in range(N_CORES)], axis=0)
    return y


# revision 10
# speedup vs baseline: 1.2115x; 1.0019x over previous
"""ChebyKAN linear layer on 8 Trainium2 NeuronCores.

Math: y[b,o] = sum_{i,d} T_d(w[b,i]) * C[i,o,d], with w = tanh(tanh(x)) and
T_d the Chebyshev polynomials (cos(d*arccos(w)) == T_d(w) for |w|<=1).

The ACT engine has no arccos/cos, so the device evaluates the Chebyshev-product
basis phi = [T1, T1^2, T1*T2, T2^2, T2*T3, T3^2, T3*T4, T4^2] built from
Square/multiply ops (T2, T4 and the T3 helper come from cheap affine ops). Via
T_{2k} = 2*T_k^2-1 and T_{m+n} = 2*T_m*T_n - T_{m-n}, an exact host-side
linear transform maps Chebyshev coefficients onto this basis with O(1)
conditioning; the constant column folds into a per-o bias added during PSUM
evacuation.

Sharding: data-parallel over batch b (16384 -> 2048/core); coeffs replicated.
x is pre-transposed on the host so the contraction dim (c_in) lands on SBUF
partitions; the kernel computes y^T per core and the host transposes back.

Everything on device is fp16: x ships as fp16 (2MB/core), weights as fp16
scaled x16 on the host (4MB/core; the raw values ~1e-4 would graze the fp16
subnormal floor, and the 1/16 rescale folds into the evacuation's activation
scale for free), and the whole basis chain computes in fp16 (10 mantissa
bits -> measured end-to-end error ~1.2e-3 vs the 2e-2 gate, better than a
single-rounded bf16 scheme). fp16 matmuls stream 1 cycle/row -- measured
213ns per 512-row matmul, the PE floor -- PSUM accumulates fp32, and the
fp16-native chain needs no separate cast ops: ACT runs the critical path
(tanh -> tanh -> Square -> affine, 4 ops/row-block) writing fp16 directly,
DVE runs the 9 off-critical multiply/affine ops at 16-bit double rate. An
8-op ACT chain ran at 92% occupancy and intermittently starved the PE, which
then bounced the HAM clock gate between 1.2/2.4GHz; gpsimd (Q7) elementwise
measured ~1.5us per [128,512] cast, 2-3x the cost model, so it gets no chain
work. PSUM evacuations are stamped one block late so ACT never runs them
when a phase boundary needs the next tanh chain (the banks aren't reused for
two phases).

Schedule (from NTFF profiles): per-core time = prologue + 109.2us of PE
streaming + drain + ~7.5us fixed NRT teardown, so the tuning targets are the
edges. The batch runs in five PSUM phases sized 256/256/512/512/512: narrow
head phases cut first-matmul latency (the 64KB first x-sliver lands ~2.5us
sooner, and the half-width tanh chain is ~2x shorter); the final phase's
evacuation overlaps the staggered oc-major matmul tail. tile_wait_until stamps per
row-block keep each in-order engine queue in consumption order (the
scheduler otherwise predicts DMA completions optimistically and head-of-line
blocks the ACT queue on a late transfer). DMA: ~620ns sequencer issue +
~1.5us doorbell-to-packet wake per transfer; a tiny bias load goes first on
the gpsimd ring to absorb the wake. Weights load as one 1MB fp16 DMA per
128-row contraction block (block 0 split j-wise on gpsimd in PE consumption
order, racing the sync-ring x slivers). Output DMAs ride the gpsimd SWDGE
ring (slow ~110GB/s but fully hidden) except the last phase's, which use the
two fast HWDGE rings (sync/scalar, ~360GB/s) to drain in ~1.5us. A short
fp32 dummy-matmul burst warms the PE HAM clock gate (1.2->2.4GHz) during the
DMA ramp.
"""

import sys

if "/opt/trn_rl_repo" not in sys.path:
    sys.path.append("/opt/trn_rl_repo")

import numpy as np

import concourse.bacc as bacc
import concourse.tile as tile
from concourse import mybir
from concourse.bass_utils import run_bass_kernel_spmd

DEGREE = 8
B, C_IN, C_OUT = 16384, 512, 512
N_CORES = 8
NB = B // N_CORES            # 2048 batch rows per core
N_IB = C_IN // 128           # 4 contraction row-blocks
N_J = DEGREE                 # basis funcs phi_1..phi_8 (constant -> bias)
F32 = mybir.dt.float32
F16 = mybir.dt.float16
W_SCALE = 16.0               # host premultiplies weights; evac scales by 1/16

# (batch offset, width) per PSUM accumulation phase: narrow head phases for a
# fast pipeline ramp-in.
PHASES = [(0, 256), (256, 256), (512, 512), (1024, 512), (1536, 512)]
assert sum(w for _, w in PHASES) == NB

_CACHE = {}


def _build():
    nc = bacc.Bacc("TRN2", target_bir_lowering=False, debug=False)
    xt = nc.dram_tensor("xt", [C_IN, NB], F16, kind="ExternalInput")
    wmat = nc.dram_tensor("wmat", [C_IN, N_J * C_OUT], F16, kind="ExternalInput")
    biasv = nc.dram_tensor("biasv", [128, 4], F32, kind="ExternalInput")
    yt = nc.dram_tensor("yt", [C_OUT, NB], F32, kind="ExternalOutput")

    Tanh = mybir.ActivationFunctionType.Tanh
    Square = mybir.ActivationFunctionType.Square
    Identity = mybir.ActivationFunctionType.Identity
    ALU_MULT = mybir.AluOpType.mult
    ALU_ADD = mybir.AluOpType.add

    with tile.TileContext(nc) as tc:
        with (
            tc.tile_pool(name="const", bufs=1) as const_pool,
            tc.tile_pool(name="wts", bufs=1) as wpool,
            tc.tile_pool(name="pows", bufs=2) as ppool,
            tc.tile_pool(name="outs", bufs=2) as opool,
            tc.tile_pool(name="psum", bufs=2, space="PSUM") as pspool,
        ):
            # PE warm-up fodder: the HAM clock gate keeps the PE at 1.2GHz
            # until ~3.4us of sustained activity; fp32 dummy matmuls (two
            # LOW/HIGH passes each, ~420ns apiece) warm it up while the
            # first DMAs are in flight. The memset rides gpsimd, whose
            # framework preamble retires earliest.
            dummy = const_pool.tile([128, 128], F32, tag="dummy")
            nc.gpsimd.memset(dummy[:], 0.0)
            cm1 = const_pool.tile([128, 1], F16, tag="cm1")
            nc.gpsimd.memset(cm1[:], -1.0)
            dps = pspool.tile([128, 512], F32, tag="ps3", name="dps")
            for _ in range(8):
                nc.tensor.matmul(
                    dps[:, 0:128], lhsT=dummy[:], rhs=dummy[:],
                    start=True, stop=True,
                )

            # Tiny bias load first on gpsimd: absorbs the ~1.5us DMA-engine
            # wake so the critical x sliver starts moving sooner.
            bias_t = const_pool.tile([128, 4], F32)
            nc.gpsimd.dma_start(out=bias_t[:], in_=biasv.ap())

            # Weights: one [128, 8*512] fp16 tile per contraction row-block.
            # Row-block 0 goes j-chunk-wise on the gpsimd (SWDGE) ring in PE
            # consumption order, concurrent with the sync-ring x slivers;
            # row-blocks 1-3 are single 1MB transfers on the sync ring.
            w_sb = []
            wt0 = wpool.tile([128, N_J * C_OUT], F16, tag="wc0", name="wc0")
            for j in range(N_J):
                nc.gpsimd.dma_start(
                    out=wt0[:, j * C_OUT : (j + 1) * C_OUT],
                    in_=wmat.ap()[0:128, j * C_OUT : (j + 1) * C_OUT],
                )
            w_sb.append(wt0)

            # Phase-0 x slivers first on the sync ring: the ib=0 sliver
            # (64KB) gates the tanh chain and therefore the first matmul.
            xlbs = []
            xlb0 = ppool.tile([128, N_IB, 256], F16, tag="xlb0", bufs=1)
            for ib in range(N_IB):
                nc.sync.dma_start(
                    out=xlb0[:, ib, :],
                    in_=xt.ap()[ib * 128 : (ib + 1) * 128, 0:256],
                )
            xlbs.append(xlb0)

            def load_xlb(pi):
                off, wd = PHASES[pi]
                xlb = ppool.tile(
                    [128, N_IB, wd], F16, tag=f"xlb{pi}", bufs=1, name=f"xlb{pi}"
                )
                nc.sync.dma_start(
                    out=xlb[:],
                    in_=xt.ap()[:, off : off + wd].rearrange(
                        "(ib p) b -> p ib b", p=128
                    ),
                )
                xlbs.append(xlb)

            load_xlb(1)
            for ib in range(1, N_IB):
                wt = wpool.tile([128, N_J * C_OUT], F16, tag=f"wc{ib}", name=f"wc{ib}")
                nc.sync.dma_start(
                    out=wt[:],
                    in_=wmat.ap()[ib * 128 : (ib + 1) * 128, :],
                )
                w_sb.append(wt)
            for pi in range(2, len(PHASES)):
                load_xlb(pi)

            def w_chunk(ib, j, oc):
                return w_sb[ib][:, j * C_OUT + oc * 128 : j * C_OUT + (oc + 1) * 128]

            # Virtual-time stamps (ms) keep each in-order engine queue in
            # consumption order; the value tracks when each row-block's
            # matmuls can actually start (10.5us prologue + PE time so far).
            t_ms = 0.0105
            for pi, (off, wd) in enumerate(PHASES):
                ps = [
                    pspool.tile([128, 512], F32, tag=f"ps{oc}", name=f"ps{oc}_{pi}")
                    for oc in range(4)
                ]
                xlb = xlbs[pi]
                for ib in range(N_IB):
                    with tc.tile_wait_until(t_ms):
                        # fp16 Chebyshev-product basis chain. Critical path
                        # (tanh -> tanh -> Square/affine) stays on ACT; DVE
                        # runs the off-critical multiplies at 16-bit 2x rate.
                        s = slice(0, wd)
                        t1 = ppool.tile([128, 512], F16, tag="t1", bufs=3)
                        f2 = ppool.tile([128, 512], F16, tag="f2", bufs=3)
                        t2 = ppool.tile([128, 512], F16, tag="t2", bufs=3)
                        u3 = ppool.tile([128, 512], F16, tag="u3", bufs=3)
                        t3 = ppool.tile([128, 512], F16, tag="t3", bufs=3)
                        f3 = ppool.tile([128, 512], F16, tag="f3", bufs=3)
                        f4 = ppool.tile([128, 512], F16, tag="f4", bufs=3)
                        t4 = ppool.tile([128, 512], F16, tag="t4", bufs=3)
                        f5 = ppool.tile([128, 512], F16, tag="f5", bufs=3)
                        f6 = ppool.tile([128, 512], F16, tag="f6", bufs=3)
                        f7 = ppool.tile([128, 512], F16, tag="f7", bufs=3)
                        f8 = ppool.tile([128, 512], F16, tag="f8", bufs=3)
                        nc.scalar.activation(xlb[:, ib, :], xlb[:, ib, :], Tanh)
                        nc.scalar.activation(t1[:, s], xlb[:, ib, :], Tanh)
                        nc.scalar.activation(f2[:, s], t1[:, s], Square)
                        nc.scalar.activation(
                            t2[:, s], f2[:, s], Identity, bias=cm1[:], scale=2.0
                        )
                        nc.vector.tensor_scalar(
                            u3[:, s], f2[:, s], 4.0, -3.0, ALU_MULT, ALU_ADD
                        )
                        nc.vector.tensor_mul(t3[:, s], t1[:, s], u3[:, s])
                        nc.vector.tensor_mul(f3[:, s], t1[:, s], t2[:, s])
                        nc.vector.tensor_mul(f4[:, s], t2[:, s], t2[:, s])
                        nc.vector.tensor_scalar(
                            t4[:, s], f4[:, s], 2.0, -1.0, ALU_MULT, ALU_ADD
                        )
                        nc.vector.tensor_mul(f5[:, s], t2[:, s], t3[:, s])
                        nc.vector.tensor_mul(f6[:, s], t3[:, s], t3[:, s])
                        nc.vector.tensor_mul(f7[:, s], t3[:, s], t4[:, s])
                        nc.vector.tensor_mul(f8[:, s], t4[:, s], t4[:, s])
                        chunks = [t1, f2, f3, f4, f5, f6, f7, f8]
                        if ib < N_IB - 1:
                            order = [(j, oc) for j in range(N_J) for oc in range(4)]
                        else:
                            # oc-major on the last row-block: accumulation
                            # groups finish staggered -> evacuation overlaps
                            # the matmul stream
                            order = [(j, oc) for oc in range(4) for j in range(N_J)]
                        for j, oc in order:
                            nc.tensor.matmul(
                                ps[oc][:, s],
                                lhsT=w_chunk(ib, j, oc),
                                rhs=chunks[j][:, s],
                                start=(ib == 0 and j == 0),
                                stop=(ib == N_IB - 1 and j == N_J - 1),
                            )
                            if ib == N_IB - 1 and j == N_J - 1:
                                osb = opool.tile(
                                    [128, 512], F32, tag=f"osb{oc}", name=f"osb{oc}"
                                )
                                final = pi == len(PHASES) - 1
                                # Delay non-final evacuations one block in
                                # virtual time: ACT otherwise runs 4 PSUM
                                # evacs exactly when the next phase's tanh
                                # chain is due, stalling the PE ~2us at every
                                # phase boundary. The banks aren't reused
                                # until two phases later (bufs=2), so the
                                # evac can ride behind the next block's chain.
                                t_evac = t_ms if final else t_ms + 0.0068
                                with tc.tile_wait_until(t_evac):
                                    nc.scalar.activation(
                                        osb[:, s], ps[oc][:, s], Identity,
                                        bias=bias_t[:, oc : oc + 1],
                                        scale=1.0 / W_SCALE,
                                    )
                                    if final:
                                        # final outputs on the fast HWDGE rings
                                        out_eng = (
                                            nc.scalar, nc.sync, nc.scalar, nc.sync
                                        )[oc]
                                    else:
                                        out_eng = nc.gpsimd
                                    out_eng.dma_start(
                                        out=yt.ap()[
                                            oc * 128 : (oc + 1) * 128, off : off + wd
                                        ],
                                        in_=osb[:, s],
                                    )
                    t_ms += wd * 128 * 0.4167 * 1e-6
    nc.compile()
    return nc


def _host_transform(cheby_coeffs):
    # Map Chebyshev coefficients onto the device phi basis:
    # phi = [T1, T1^2, T1*T2, T2^2, T2*T3, T3^2, T3*T4, T4^2] and a constant.
    # T_{2k} = 2*T_k^2 - 1, T_{m+n} = 2*T_m*T_n - T_{m-n} =>
    #   y = bias + (C1-C3-C5-C7)*T1 + sum_{d=2..8} 2*C_d * phi_{d-1}
    #   bias_o = sum_i (C0 - C2 - C4 - C6 - C8)
    C64 = cheby_coeffs.astype(np.float64)
    bias = (C64[..., 0] - C64[..., 2] - C64[..., 4] - C64[..., 6] - C64[..., 8]).sum(
        axis=0
    )
    W = np.empty((C_IN, C_OUT, N_J), np.float64)
    W[..., 0] = C64[..., 1] - C64[..., 3] - C64[..., 5] - C64[..., 7]
    for d in range(2, DEGREE + 1):
        W[..., d - 1] = 2.0 * C64[..., d]
    # [i, j*512+o]: per-partition-contiguous coefficient rows; fp16 on device,
    # premultiplied by W_SCALE (undone by the evacuation's activation scale)
    # to clear the fp16 subnormal floor.
    Wd = np.ascontiguousarray(
        (W.transpose(0, 2, 1).reshape(C_IN, N_J * C_OUT) * W_SCALE).astype(np.float16)
    )
    bias_dev = np.ascontiguousarray(bias.reshape(4, 128).T.astype(np.float32))
    return Wd, bias_dev


def kernel(x, cheby_coeffs):
    x = np.asarray(x, dtype=np.float32)
    cheby_coeffs = np.asarray(cheby_coeffs, dtype=np.float32)
    if "nc" not in _CACHE:
        _CACHE["nc"] = _build()
    nc = _CACHE["nc"]

    Wd, bias_dev = _host_transform(cheby_coeffs)
    xT = np.ascontiguousarray(x.T)                       # [c_in, b]
    in_maps = [
        {
            "xt": np.ascontiguousarray(xT[:, c * NB : (c + 1) * NB].astype(np.float16)),
            "wmat": Wd,
            "biasv": bias_dev,
        }
        for c in range(N_CORES)
    ]
    res = run_bass_kernel_spmd(nc, in_maps, core_ids=list(range(N_CORES)))
    y = np.concatenate([res.results[c]["yt"].T for c in range(N_CORES)], axis=0)
    return y


# revision 12
# speedup vs baseline: 1.3345x; 1.1015x over previous
"""ChebyKAN linear layer on 8 Trainium2 NeuronCores.

Math: y[b,o] = sum_{i,d} T_d(w[b,i]) * C[i,o,d], with w = tanh(tanh(x)) and
T_d the Chebyshev polynomials (cos(d*arccos(w)) == T_d(w) for |w|<=1).

The ACT engine has no arccos/cos, so the device evaluates the Chebyshev-product
basis phi = [T1, T1^2, T1*T2, T2^2, T2*T3, T3^2, T3*T4, T4^2] built from
Square/multiply ops (T2, T4 and the T3 helper come from cheap affine ops). Via
T_{2k} = 2*T_k^2-1 and T_{m+n} = 2*T_m*T_n - T_{m-n}, an exact host-side
linear transform maps Chebyshev coefficients onto this basis with O(1)
conditioning; the constant column folds into a per-o bias added during PSUM
evacuation (shipped x16; the host divides the gathered output by 16, exact in
fp32).

Sharding: data-parallel over batch b (16384 -> 2048/core); coeffs replicated.
x is pre-transposed on the host so the contraction dim (c_in) lands on SBUF
partitions; the kernel computes y^T per core and the host transposes back.

Everything on device is fp16: x ships as fp16 (2MB/core), weights as fp16
scaled x16 on the host (4MB/core; the raw values ~1e-4 would graze the fp16
subnormal floor), and the whole basis chain computes in fp16 (10 mantissa
bits -> measured end-to-end error ~1.2e-3 vs the 2e-2 gate). fp16 matmuls
stream 1 cycle/row -- measured 213ns per 512-row matmul, the PE floor --
PSUM accumulates fp32, and the fp16-native chain needs no cast ops: ACT runs
the critical path (tanh -> tanh -> Square -> affine, 4 ops/row-block), DVE
runs the 9 off-critical multiply/affine ops at 16-bit double rate.

Scheduling lessons baked in (from NTFF profiles of prior revisions): the
engine queues are strictly in-order, so EMISSION order is the schedule.
(1) PSUM evacuations for phase p are emitted after phase p+1's first
row-block: emitted any earlier they head-of-line block the ACT/DVE queues
waiting for the accumulation group to stop (~2-3us PE stall per phase
boundary, which also drops the PE HAM clock gate 2.4->1.2GHz and bleeds
another ~3us); the banks are not reused until phase p+2 (bufs=2), so late
evacuation is free. Evacs alternate ACT/DVE (bias applied via per-partition
tensor_scalar AP operand). (2) All input DMAs ride the sync HWDGE ring in
exact consumption order -- the 16 DMA engines round-robin ACTIVE transfers,
so issuing a late-needed 1MB load early steals bandwidth from the critical
first tiles (measured: gpsimd SWDGE delivers only ~110GB/s per queue and
serialized the row-block-0 weight chunks, starving LDWEIGHTS ~10us).
(3) gpsimd (Q7) gets no elementwise work (measured ~1.5us per [128,512]
cast, 2-3x the cost model) and no PSUM access; it only issues the hidden
mid-phase output DMAs. The final phase's outputs use the two fast HWDGE
rings (sync/scalar, ~360GB/s). (4) The batch runs in five PSUM phases sized
256/256/512/512/512: narrow head phases cut first-matmul latency (the 64KB
first x-sliver lands sooner and the half-width tanh chain is ~2x shorter).
(5) A ~3.4us burst of fp32 dummy matmuls (~420ns apiece) warms the PE HAM
clock gate during the DMA ramp; a tiny bias load goes first on the gpsimd
ring to absorb the ~1.5us DMA-engine doorbell-to-packet wake. (6) Narrow
tile_wait_until stamps per row-block bias the Tile scheduler toward
consumption order (its DMA-completion predictions are optimistic).
"""

import sys

if "/opt/trn_rl_repo" not in sys.path:
    sys.path.append("/opt/trn_rl_repo")

import numpy as np

import concourse.bacc as bacc
import concourse.tile as tile
from concourse import mybir
from concourse.bass_utils import run_bass_kernel_spmd

DEGREE = 8
B, C_IN, C_OUT = 16384, 512, 512
N_CORES = 8
NB = B // N_CORES            # 2048 batch rows per core
N_IB = C_IN // 128           # 4 contraction row-blocks
N_J = DEGREE                 # basis funcs phi_1..phi_8 (constant -> bias)
F32 = mybir.dt.float32
F16 = mybir.dt.float16
W_SCALE = 16.0               # host premultiplies weights+bias; host divides y

# (batch offset, width) per PSUM accumulation phase: narrow head phases for a
# fast pipeline ramp-in.
PHASES = [(0, 256), (256, 256), (512, 512), (1024, 512), (1536, 512)]
assert sum(w for _, w in PHASES) == NB

_CACHE = {}


def _build():
    nc = bacc.Bacc("TRN2", target_bir_lowering=False, debug=False)
    xt = nc.dram_tensor("xt", [C_IN, NB], F16, kind="ExternalInput")
    wmat = nc.dram_tensor("wmat", [C_IN, N_J * C_OUT], F16, kind="ExternalInput")
    biasv = nc.dram_tensor("biasv", [128, 4], F32, kind="ExternalInput")
    yt = nc.dram_tensor("yt", [C_OUT, NB], F32, kind="ExternalOutput")

    Tanh = mybir.ActivationFunctionType.Tanh
    Square = mybir.ActivationFunctionType.Square
    Identity = mybir.ActivationFunctionType.Identity
    ALU_MULT = mybir.AluOpType.mult
    ALU_ADD = mybir.AluOpType.add

    with tile.TileContext(nc) as tc:
        with (
            tc.tile_pool(name="const", bufs=1) as const_pool,
            tc.tile_pool(name="wts", bufs=1) as wpool,
            tc.tile_pool(name="pows", bufs=2) as ppool,
            tc.tile_pool(name="outs", bufs=2) as opool,
            tc.tile_pool(name="psum", bufs=2, space="PSUM") as pspool,
        ):
            # PE warm-up fodder while the first DMAs are in flight.
            dummy = const_pool.tile([128, 128], F32, tag="dummy")
            nc.gpsimd.memset(dummy[:], 0.0)
            cm1 = const_pool.tile([128, 1], F16, tag="cm1")
            nc.gpsimd.memset(cm1[:], -1.0)
            dps = pspool.tile([128, 512], F32, tag="ps3", name="dps")
            for _ in range(8):
                nc.tensor.matmul(
                    dps[:, 0:128], lhsT=dummy[:], rhs=dummy[:],
                    start=True, stop=True,
                )

            # Tiny bias load first on gpsimd: absorbs the DMA-engine wake.
            bias_t = const_pool.tile([128, 4], F32)
            nc.gpsimd.dma_start(out=bias_t[:], in_=biasv.ap())

            # All input loads on the sync HWDGE ring in consumption order.
            w_sb = [
                wpool.tile([128, N_J * C_OUT], F16, tag=f"wc{ib}", name=f"wc{ib}")
                for ib in range(N_IB)
            ]
            xlb0 = ppool.tile([128, N_IB, 256], F16, tag="xlb0", bufs=1)
            xlbs = [xlb0]
            # 64KB ib=0 sliver gates the tanh chain -> first matmul
            nc.sync.dma_start(out=xlb0[:, 0, :], in_=xt.ap()[0:128, 0:256])
            nc.sync.dma_start(
                out=w_sb[0][:, 0:C_OUT], in_=wmat.ap()[0:128, 0:C_OUT]
            )
            nc.sync.dma_start(
                out=w_sb[0][:, C_OUT : 2 * C_OUT],
                in_=wmat.ap()[0:128, C_OUT : 2 * C_OUT],
            )
            nc.sync.dma_start(
                out=xlb0[:, 1:N_IB, :],
                in_=xt.ap()[128:C_IN, 0:256].rearrange("(ib p) b -> p ib b", p=128),
            )
            for j in range(2, N_J):
                nc.sync.dma_start(
                    out=w_sb[0][:, j * C_OUT : (j + 1) * C_OUT],
                    in_=wmat.ap()[0:128, j * C_OUT : (j + 1) * C_OUT],
                )

            def load_xlb(pi):
                off, wd = PHASES[pi]
                xlb = ppool.tile(
                    [128, N_IB, wd], F16, tag=f"xlb{pi}", bufs=1, name=f"xlb{pi}"
                )
                nc.sync.dma_start(
                    out=xlb[:],
                    in_=xt.ap()[:, off : off + wd].rearrange(
                        "(ib p) b -> p ib b", p=128
                    ),
                )
                xlbs.append(xlb)

            load_xlb(1)
            for ib in range(1, N_IB):
                nc.sync.dma_start(
                    out=w_sb[ib][:], in_=wmat.ap()[ib * 128 : (ib + 1) * 128, :]
                )
            for pi in range(2, len(PHASES)):
                load_xlb(pi)

            def w_chunk(ib, j, oc):
                return w_sb[ib][:, j * C_OUT + oc * 128 : j * C_OUT + (oc + 1) * 128]

            def emit_evacs(pending):
                # Evacuate phase p's PSUM banks; called after phase p+1's
                # first row-block so the in-order ACT/DVE queues never wait
                # on a still-accumulating group. Alternate ACT/DVE.
                ps, off, wd, final = pending
                s = slice(0, wd)
                for oc in range(4):
                    osb = opool.tile(
                        [128, 512], F32, tag=f"osb{oc}", name=f"osb{oc}"
                    )
                    if oc % 2 == 0:
                        nc.scalar.activation(
                            osb[:, s], ps[oc][:, s], Identity,
                            bias=bias_t[:, oc : oc + 1],
                        )
                    else:
                        nc.vector.tensor_scalar(
                            osb[:, s], ps[oc][:, s], bias_t[:, oc : oc + 1],
                            None, ALU_ADD,
                        )
                    if final:
                        out_eng = (nc.scalar, nc.sync, nc.scalar, nc.sync)[oc]
                    else:
                        out_eng = nc.gpsimd
                    out_eng.dma_start(
                        out=yt.ap()[oc * 128 : (oc + 1) * 128, off : off + wd],
                        in_=osb[:, s],
                    )

            # Virtual-time stamps (ms): when each row-block's matmuls can
            # start (10.5us prologue + PE time so far).
            t_ms = 0.0105
            pending_evac = None
            for pi, (off, wd) in enumerate(PHASES):
                ps = [
                    pspool.tile([128, 512], F32, tag=f"ps{oc}", name=f"ps{oc}_{pi}")
                    for oc in range(4)
                ]
                xlb = xlbs[pi]
                for ib in range(N_IB):
                    with tc.tile_wait_until(t_ms):
                        # fp16 Chebyshev-product basis chain. Critical path
                        # (tanh -> tanh -> Square -> affine) on ACT; DVE runs
                        # the off-critical ops at 16-bit 2x rate.
                        s = slice(0, wd)
                        t1 = ppool.tile([128, 512], F16, tag="t1", bufs=3)
                        f2 = ppool.tile([128, 512], F16, tag="f2", bufs=3)
                        t2 = ppool.tile([128, 512], F16, tag="t2", bufs=3)
                        u3 = ppool.tile([128, 512], F16, tag="u3", bufs=3)
                        t3 = ppool.tile([128, 512], F16, tag="t3", bufs=3)
                        f3 = ppool.tile([128, 512], F16, tag="f3", bufs=3)
                        f4 = ppool.tile([128, 512], F16, tag="f4", bufs=3)
                        t4 = ppool.tile([128, 512], F16, tag="t4", bufs=3)
                        f5 = ppool.tile([128, 512], F16, tag="f5", bufs=3)
                        f6 = ppool.tile([128, 512], F16, tag="f6", bufs=3)
                        f7 = ppool.tile([128, 512], F16, tag="f7", bufs=3)
                        f8 = ppool.tile([128, 512], F16, tag="f8", bufs=3)
                        nc.scalar.activation(xlb[:, ib, :], xlb[:, ib, :], Tanh)
                        nc.scalar.activation(t1[:, s], xlb[:, ib, :], Tanh)
                        nc.scalar.activation(f2[:, s], t1[:, s], Square)
                        nc.scalar.activation(
                            t2[:, s], f2[:, s], Identity, bias=cm1[:], scale=2.0
                        )
                        nc.vector.tensor_scalar(
                            u3[:, s], f2[:, s], 4.0, -3.0, ALU_MULT, ALU_ADD
                        )
                        nc.vector.tensor_mul(t3[:, s], t1[:, s], u3[:, s])
                        nc.vector.tensor_mul(f3[:, s], t1[:, s], t2[:, s])
                        nc.vector.tensor_mul(f4[:, s], t2[:, s], t2[:, s])
                        nc.vector.tensor_scalar(
                            t4[:, s], f4[:, s], 2.0, -1.0, ALU_MULT, ALU_ADD
                        )
                        nc.vector.tensor_mul(f5[:, s], t2[:, s], t3[:, s])
                        nc.vector.tensor_mul(f6[:, s], t3[:, s], t3[:, s])
                        nc.vector.tensor_mul(f7[:, s], t3[:, s], t4[:, s])
                        nc.vector.tensor_mul(f8[:, s], t4[:, s], t4[:, s])
                        chunks = [t1, f2, f3, f4, f5, f6, f7, f8]
                        if ib < N_IB - 1:
                            order = [(j, oc) for j in range(N_J) for oc in range(4)]
                        else:
                            # oc-major on the last row-block: accumulation
                            # groups finish staggered
                            order = [(j, oc) for oc in range(4) for j in range(N_J)]
                        for j, oc in order:
                            nc.tensor.matmul(
                                ps[oc][:, s],
                                lhsT=w_chunk(ib, j, oc),
                                rhs=chunks[j][:, s],
                                start=(ib == 0 and j == 0),
                                stop=(ib == N_IB - 1 and j == N_J - 1),
                            )
                        if ib == 0 and pending_evac is not None:
                            emit_evacs(pending_evac)
                            pending_evac = None
                    t_ms += wd * 128 * 0.4167 * 1e-6
                final = pi == len(PHASES) - 1
                if final:
                    with tc.tile_wait_until(t_ms):
                        emit_evacs((ps, off, wd, True))
                else:
                    pending_evac = (ps, off, wd, False)
    nc.compile()
    return nc


def _host_transform(cheby_coeffs):
    # Map Chebyshev coefficients onto the device phi basis:
    # phi = [T1, T1^2, T1*T2, T2^2, T2*T3, T3^2, T3*T4, T4^2] and a constant.
    # T_{2k} = 2*T_k^2 - 1, T_{m+n} = 2*T_m*T_n - T_{m-n} =>
    #   y = bias + (C1-C3-C5-C7)*T1 + sum_{d=2..8} 2*C_d * phi_{d-1}
    #   bias_o = sum_i (C0 - C2 - C4 - C6 - C8)
    C64 = cheby_coeffs.astype(np.float64)
    bias = (C64[..., 0] - C64[..., 2] - C64[..., 4] - C64[..., 6] - C64[..., 8]).sum(
        axis=0
    )
    W = np.empty((C_IN, C_OUT, N_J), np.float64)
    W[..., 0] = C64[..., 1] - C64[..., 3] - C64[..., 5] - C64[..., 7]
    for d in range(2, DEGREE + 1):
        W[..., d - 1] = 2.0 * C64[..., d]
    # [i, j*512+o]: per-partition-contiguous coefficient rows; fp16 on device,
    # premultiplied by W_SCALE (undone on the host) to clear the fp16
    # subnormal floor. The bias is folded in at the same scale.
    Wd = np.ascontiguousarray(
        (W.transpose(0, 2, 1).reshape(C_IN, N_J * C_OUT) * W_SCALE).astype(np.float16)
    )
    bias_dev = np.ascontiguousarray(
        (bias * W_SCALE).reshape(4, 128).T.astype(np.float32)
    )
    return Wd, bias_dev


def make_in_maps(x, cheby_coeffs):
    Wd, bias_dev = _host_transform(cheby_coeffs)
    xT = np.ascontiguousarray(x.T)                       # [c_in, b]
    return [
        {
            "xt": np.ascontiguousarray(xT[:, c * NB : (c + 1) * NB].astype(np.float16)),
            "wmat": Wd,
            "biasv": bias_dev,
        }
        for c in range(N_CORES)
    ]


def kernel(x, cheby_coeffs):
    x = np.asarray(x, dtype=np.float32)
    cheby_coeffs = np.asarray(cheby_coeffs, dtype=np.float32)
    if "nc" not in _CACHE:
        _CACHE["nc"] = _build()
    nc = _CACHE["nc"]

    in_maps = make_in_maps(x, cheby_coeffs)
    res = run_bass_kernel_spmd(nc, in_maps, core_ids=list(range(N_CORES)))
    y = np.concatenate([res.results[c]["yt"].T for c in range(N_CORES)], axis=0)
    return (y * np.float32(1.0 / W_SCALE)).astype(np.float32)


# revision 13
# speedup vs baseline: 1.3545x; 1.0150x over previous
"""ChebyKAN linear layer on 8 Trainium2 NeuronCores.

Math: y[b,o] = sum_{i,d} T_d(w[b,i]) * C[i,o,d], with w = tanh(tanh(x)) and
T_d the Chebyshev polynomials (cos(d*arccos(w)) == T_d(w) for |w|<=1).

The ACT engine has no arccos/cos, so the device evaluates the Chebyshev-product
basis phi = [T1, T1^2, T1*T2, T2^2, T2*T3, T3^2, T3*T4, T4^2] built from
Square/multiply ops (T2, T4 and the T3 helper come from cheap affine ops). Via
T_{2k} = 2*T_k^2-1 and T_{m+n} = 2*T_m*T_n - T_{m-n}, an exact host-side
linear transform maps Chebyshev coefficients onto this basis with O(1)
conditioning; the constant column folds into a per-o bias added during PSUM
evacuation (shipped x16; the host divides the gathered output by 16, exact in
fp32).

Sharding: data-parallel over batch b (16384 -> 2048/core); coeffs replicated.
x is pre-transposed on the host so the contraction dim (c_in) lands on SBUF
partitions; the kernel computes y^T per core and the host transposes back.

Everything on device is fp16: x ships as fp16 (2MB/core), weights as fp16
scaled x16 on the host (4MB/core; the raw values ~1e-4 would graze the fp16
subnormal floor), and the whole basis chain computes in fp16 (10 mantissa
bits -> measured end-to-end error ~1.2e-3 vs the 2e-2 gate). fp16 matmuls
stream 1 cycle/row -- measured 213ns per 512-row matmul, the PE floor --
PSUM accumulates fp32, and the fp16-native chain needs no cast ops: ACT runs
the critical path (tanh -> tanh -> Square -> affine, 4 ops/row-block), DVE
runs the 9 off-critical multiply/affine ops at 16-bit double rate.

Batch schedule: 2048 rows/core = two interleaved 256-wide PSUM phases (8
banks, processed row-block by row-block so each 1MB weight block serves both
phases before the next is needed -- a single narrow phase demands weights at
300GB/s exactly during the DMA ramp and stalls LDWEIGHTS) followed by three
512-wide phases. The narrow head still buys the fast ramp-in: the 64KB first
x-sliver lands ~2.5us sooner than a 256KB one and the half-width tanh chain
halves the first-matmul latency.

Scheduling lessons baked in (from NTFF profiles of prior revisions): the
engine queues are strictly in-order, so EMISSION order is the schedule.
(1) PSUM evacuations for a phase are emitted after the NEXT phase's first
row-block: any earlier they head-of-line block the ACT/DVE queues waiting
for the accumulation group to stop (~2-3us PE stall per phase boundary,
which also drops the PE HAM clock gate 2.4->1.2GHz and bleeds another ~3us);
the banks are not reused until one more phase later (bufs=2), so late
evacuation is free. Evacs alternate ACT/DVE (bias via the per-partition AP
scalar of tensor_scalar). (2) All input DMAs ride the sync HWDGE ring in
exact consumption order -- the 16 DMA engines round-robin ACTIVE transfers,
so issuing a late-needed 1MB load early steals bandwidth from the critical
first tiles (measured: gpsimd SWDGE delivers only ~110GB/s per queue and
serialized the row-block-0 weight chunks, starving LDWEIGHTS ~10us).
(3) gpsimd (Q7) gets no elementwise work (measured ~1.5us per [128,512]
cast, 2-3x the cost model) and no PSUM access; it only issues the hidden
mid-phase output DMAs. The final phase's outputs use the two fast HWDGE
rings (sync/scalar, ~360GB/s). (4) A ~3.4us burst of fp32 dummy matmuls
warms the PE HAM clock gate during the DMA ramp; a tiny bias load goes first
on the gpsimd ring to absorb the ~1.5us DMA-engine doorbell-to-packet wake.
(5) tile_wait_until stamps per row-block bias the Tile scheduler toward
consumption order (its DMA-completion predictions are optimistic).
"""

import sys

if "/opt/trn_rl_repo" not in sys.path:
    sys.path.append("/opt/trn_rl_repo")

import numpy as np

import concourse.bacc as bacc
import concourse.tile as tile
from concourse import mybir
from concourse.bass_utils import run_bass_kernel_spmd

DEGREE = 8
B, C_IN, C_OUT = 16384, 512, 512
N_CORES = 8
NB = B // N_CORES            # 2048 batch rows per core
N_IB = C_IN // 128           # 4 contraction row-blocks
N_J = DEGREE                 # basis funcs phi_1..phi_8 (constant -> bias)
F32 = mybir.dt.float32
F16 = mybir.dt.float16
W_SCALE = 16.0               # host premultiplies weights+bias; host divides y

PAIR = [(0, 256), (256, 256)]                       # interleaved narrow phases
WIDE = [(512, 512), (1024, 512), (1536, 512)]
assert sum(w for _, w in PAIR + WIDE) == NB

_CACHE = {}


def _build():
    nc = bacc.Bacc("TRN2", target_bir_lowering=False, debug=False)
    xt = nc.dram_tensor("xt", [C_IN, NB], F16, kind="ExternalInput")
    wmat = nc.dram_tensor("wmat", [C_IN, N_J * C_OUT], F16, kind="ExternalInput")
    biasv = nc.dram_tensor("biasv", [128, 4], F32, kind="ExternalInput")
    yt = nc.dram_tensor("yt", [C_OUT, NB], F32, kind="ExternalOutput")

    Tanh = mybir.ActivationFunctionType.Tanh
    Square = mybir.ActivationFunctionType.Square
    Identity = mybir.ActivationFunctionType.Identity
    ALU_MULT = mybir.AluOpType.mult
    ALU_ADD = mybir.AluOpType.add

    with tile.TileContext(nc) as tc:
        with (
            tc.tile_pool(name="const", bufs=1) as const_pool,
            tc.tile_pool(name="wts", bufs=1) as wpool,
            tc.tile_pool(name="pows", bufs=2) as ppool,
            tc.tile_pool(name="outs", bufs=2) as opool,
            tc.tile_pool(name="psum", bufs=2, space="PSUM") as pspool,
        ):
            # PE warm-up fodder while the first DMAs are in flight.
            dummy = const_pool.tile([128, 128], F32, tag="dummy")
            nc.gpsimd.memset(dummy[:], 0.0)
            cm1 = const_pool.tile([128, 1], F16, tag="cm1")
            nc.gpsimd.memset(cm1[:], -1.0)
            dps = pspool.tile([128, 512], F32, tag="ps3", name="dps")
            for _ in range(8):
                nc.tensor.matmul(
                    dps[:, 0:128], lhsT=dummy[:], rhs=dummy[:],
                    start=True, stop=True,
                )

            # Tiny bias load first on gpsimd: absorbs the DMA-engine wake.
            bias_t = const_pool.tile([128, 4], F32)
            nc.gpsimd.dma_start(out=bias_t[:], in_=biasv.ap())

            # All input loads on the sync HWDGE ring in consumption order.
            w_sb = [
                wpool.tile([128, N_J * C_OUT], F16, tag=f"wc{ib}", name=f"wc{ib}")
                for ib in range(N_IB)
            ]

            def load_w(ib, j0, j1):
                nc.sync.dma_start(
                    out=w_sb[ib][:, j0 * C_OUT : j1 * C_OUT],
                    in_=wmat.ap()[ib * 128 : (ib + 1) * 128, j0 * C_OUT : j1 * C_OUT],
                )

            xlb0 = ppool.tile([128, N_IB, 256], F16, tag="xlb0", bufs=1)
            xlb1 = ppool.tile([128, N_IB, 256], F16, tag="xlb1", bufs=1)
            xlbs = [xlb0, xlb1]
            # 64KB (ph0, ib0) sliver gates the tanh chain -> first matmul
            nc.sync.dma_start(out=xlb0[:, 0, :], in_=xt.ap()[0:128, 0:256])
            load_w(0, 0, 1)
            load_w(0, 1, 2)
            nc.sync.dma_start(
                out=xlb1[:],
                in_=xt.ap()[:, 256:512].rearrange("(ib p) b -> p ib b", p=128),
            )
            nc.sync.dma_start(
                out=xlb0[:, 1:N_IB, :],
                in_=xt.ap()[128:C_IN, 0:256].rearrange("(ib p) b -> p ib b", p=128),
            )
            load_w(0, 2, 3)
            load_w(0, 3, 4)
            load_w(0, 4, 5)
            load_w(1, 0, 4)
            load_w(0, 5, 6)
            load_w(0, 6, 7)
            load_w(0, 7, 8)
            load_w(1, 4, 8)

            def load_xlb(pi_wide):
                off, wd = WIDE[pi_wide]
                xlb = ppool.tile(
                    [128, N_IB, wd], F16, tag=f"xw{pi_wide}", bufs=1,
                    name=f"xw{pi_wide}",
                )
                nc.sync.dma_start(
                    out=xlb[:],
                    in_=xt.ap()[:, off : off + wd].rearrange(
                        "(ib p) b -> p ib b", p=128
                    ),
                )
                xlbs.append(xlb)

            load_w(2, 0, 8)
            load_xlb(0)
            load_w(3, 0, 8)
            load_xlb(1)
            load_xlb(2)

            def w_chunk(ib, j, oc):
                return w_sb[ib][:, j * C_OUT + oc * 128 : j * C_OUT + (oc + 1) * 128]

            def emit_chain_and_mms(ps, xlb, ib, wd, start_ib, stop_ib):
                # fp16 Chebyshev-product basis chain. Critical path
                # (tanh -> tanh -> Square -> affine) on ACT; DVE runs the
                # off-critical ops at 16-bit 2x rate.
                s = slice(0, wd)
                t1 = ppool.tile([128, 512], F16, tag="t1", bufs=4)
                f2 = ppool.tile([128, 512], F16, tag="f2", bufs=4)
                t2 = ppool.tile([128, 512], F16, tag="t2", bufs=4)
                u3 = ppool.tile([128, 512], F16, tag="u3", bufs=4)
                t3 = ppool.tile([128, 512], F16, tag="t3", bufs=4)
                f3 = ppool.tile([128, 512], F16, tag="f3", bufs=4)
                f4 = ppool.tile([128, 512], F16, tag="f4", bufs=4)
                t4 = ppool.tile([128, 512], F16, tag="t4", bufs=4)
                f5 = ppool.tile([128, 512], F16, tag="f5", bufs=4)
                f6 = ppool.tile([128, 512], F16, tag="f6", bufs=4)
                f7 = ppool.tile([128, 512], F16, tag="f7", bufs=4)
                f8 = ppool.tile([128, 512], F16, tag="f8", bufs=4)
                nc.scalar.activation(xlb[:, ib, :], xlb[:, ib, :], Tanh)
                nc.scalar.activation(t1[:, s], xlb[:, ib, :], Tanh)
                nc.scalar.activation(f2[:, s], t1[:, s], Square)
                nc.scalar.activation(
                    t2[:, s], f2[:, s], Identity, bias=cm1[:], scale=2.0
                )
                nc.vector.tensor_scalar(
                    u3[:, s], f2[:, s], 4.0, -3.0, ALU_MULT, ALU_ADD
                )
                nc.vector.tensor_mul(t3[:, s], t1[:, s], u3[:, s])
                nc.vector.tensor_mul(f3[:, s], t1[:, s], t2[:, s])
                nc.vector.tensor_mul(f4[:, s], t2[:, s], t2[:, s])
                nc.vector.tensor_scalar(
                    t4[:, s], f4[:, s], 2.0, -1.0, ALU_MULT, ALU_ADD
                )
                nc.vector.tensor_mul(f5[:, s], t2[:, s], t3[:, s])
                nc.vector.tensor_mul(f6[:, s], t3[:, s], t3[:, s])
                nc.vector.tensor_mul(f7[:, s], t3[:, s], t4[:, s])
                nc.vector.tensor_mul(f8[:, s], t4[:, s], t4[:, s])
                chunks = [t1, f2, f3, f4, f5, f6, f7, f8]
                if ib < N_IB - 1:
                    order = [(j, oc) for j in range(N_J) for oc in range(4)]
                else:
                    # oc-major on the last row-block: groups finish staggered
                    order = [(j, oc) for oc in range(4) for j in range(N_J)]
                for j, oc in order:
                    nc.tensor.matmul(
                        ps[oc][:, s],
                        lhsT=w_chunk(ib, j, oc),
                        rhs=chunks[j][:, s],
                        start=(ib == start_ib and j == 0),
                        stop=(ib == stop_ib and j == N_J - 1),
                    )

            def emit_evacs(ps, off, wd, final):
                # Evacuate a completed phase's PSUM banks; called after the
                # NEXT phase's first row-block so the in-order ACT/DVE
                # queues never wait on a still-accumulating group.
                s = slice(0, wd)
                for oc in range(4):
                    osb = opool.tile(
                        [128, 512], F32, tag=f"osb{oc}", name=f"osb{oc}"
                    )
                    if oc % 2 == 0:
                        nc.scalar.activation(
                            osb[:, s], ps[oc][:, s], Identity,
                            bias=bias_t[:, oc : oc + 1],
                        )
                    else:
                        nc.vector.tensor_scalar(
                            osb[:, s], ps[oc][:, s], bias_t[:, oc : oc + 1],
                            None, ALU_ADD,
                        )
                    if final:
                        out_eng = (nc.scalar, nc.sync, nc.scalar, nc.sync)[oc]
                    else:
                        out_eng = nc.gpsimd
                    out_eng.dma_start(
                        out=yt.ap()[oc * 128 : (oc + 1) * 128, off : off + wd],
                        in_=osb[:, s],
                    )

            # Virtual-time stamps (ms): when each row-block's matmuls can
            # start (10.5us prologue + PE time so far).
            t_ms = 0.0105

            # --- interleaved narrow pair ---
            ps_pair = [
                [
                    pspool.tile(
                        [128, 512], F32, tag=f"ps{oc}", name=f"ps{oc}_p{phi}"
                    )
                    for oc in range(4)
                ]
                for phi in range(2)
            ]
            for ib in range(N_IB):
                for phi, (off, wd) in enumerate(PAIR):
                    with tc.tile_wait_until(t_ms):
                        emit_chain_and_mms(
                            ps_pair[phi], xlbs[phi], ib, wd, 0, N_IB - 1
                        )
                    t_ms += wd * 128 * 0.4167 * 1e-6
            pending = [(ps_pair[0], PAIR[0][0], 256), (ps_pair[1], PAIR[1][0], 256)]

            # --- wide phases ---
            for pi, (off, wd) in enumerate(WIDE):
                ps = [
                    pspool.tile([128, 512], F32, tag=f"ps{oc}", name=f"ps{oc}_w{pi}")
                    for oc in range(4)
                ]
                xlb = xlbs[2 + pi]
                final = pi == len(WIDE) - 1
                for ib in range(N_IB):
                    with tc.tile_wait_until(t_ms):
                        emit_chain_and_mms(ps, xlb, ib, wd, 0, N_IB - 1)
                        if ib == 0:
                            for pps, poff, pwd in pending:
                                emit_evacs(pps, poff, pwd, False)
                            pending = []
                    t_ms += wd * 128 * 0.4167 * 1e-6
                if final:
                    with tc.tile_wait_until(t_ms):
                        emit_evacs(ps, off, wd, True)
                else:
                    pending = [(ps, off, wd)]
    nc.compile()
    return nc


def _host_transform(cheby_coeffs):
    # Map Chebyshev coefficients onto the device phi basis:
    # phi = [T1, T1^2, T1*T2, T2^2, T2*T3, T3^2, T3*T4, T4^2] and a constant.
    # T_{2k} = 2*T_k^2 - 1, T_{m+n} = 2*T_m*T_n - T_{m-n} =>
    #   y = bias + (C1-C3-C5-C7)*T1 + sum_{d=2..8} 2*C_d * phi_{d-1}
    #   bias_o = sum_i (C0 - C2 - C4 - C6 - C8)
    C64 = cheby_coeffs.astype(np.float64)
    bias = (C64[..., 0] - C64[..., 2] - C64[..., 4] - C64[..., 6] - C64[..., 8]).sum(
        axis=0
    )
    W = np.empty((C_IN, C_OUT, N_J), np.float64)
    W[..., 0] = C64[..., 1] - C64[..., 3] - C64[..., 5] - C64[..., 7]
    for d in range(2, DEGREE + 1):
        W[..., d - 1] = 2.0 * C64[..., d]
    # [i, j*512+o]: per-partition-contiguous coefficient rows; fp16 on device,
    # premultiplied by W_SCALE (undone on the host) to clear the fp16
    # subnormal floor. The bias is folded in at the same scale.
    Wd = np.ascontiguousarray(
        (W.transpose(0, 2, 1).reshape(C_IN, N_J * C_OUT) * W_SCALE).astype(np.float16)
    )
    bias_dev = np.ascontiguousarray(
        (bias * W_SCALE).reshape(4, 128).T.astype(np.float32)
    )
    return Wd, bias_dev


def make_in_maps(x, cheby_coeffs):
    Wd, bias_dev = _host_transform(cheby_coeffs)
    xT = np.ascontiguousarray(x.T)                       # [c_in, b]
    return [
        {
            "xt": np.ascontiguousarray(xT[:, c * NB : (c + 1) * NB].astype(np.float16)),
            "wmat": Wd,
            "biasv": bias_dev,
        }
        for c in range(N_CORES)
    ]


def kernel(x, cheby_coeffs):
    x = np.asarray(x, dtype=np.float32)
    cheby_coeffs = np.asarray(cheby_coeffs, dtype=np.float32)
    if "nc" not in _CACHE:
        _CACHE["nc"] = _build()
    nc = _CACHE["nc"]

    in_maps = make_in_maps(x, cheby_coeffs)
    res = run_bass_kernel_spmd(nc, in_maps, core_ids=list(range(N_CORES)))
    y = np.concatenate([res.results[c]["yt"].T for c in range(N_CORES)], axis=0)
    return (y * np.float32(1.0 / W_SCALE)).astype(np.float32)


# revision 20
# speedup vs baseline: 1.3971x; 1.0314x over previous
"""ChebyKAN linear layer on 8 Trainium2 NeuronCores.

Math: y[b,o] = sum_{i,d} T_d(w[b,i]) * C[i,o,d], with w = tanh(tanh(x)) and
T_d the Chebyshev polynomials (cos(d*arccos(w)) == T_d(w) for |w|<=1).

The ACT engine has no arccos/cos, so the device evaluates the Chebyshev-product
basis phi = [T1, T1^2, T1*T2, T2^2, T2*T3, T3^2, T3*T4, T4^2] built from
Square/multiply ops (T2, T4 and the T3 helper come from cheap affine ops). Via
T_{2k} = 2*T_k^2-1 and T_{m+n} = 2*T_m*T_n - T_{m-n}, an exact host-side
linear transform maps Chebyshev coefficients onto this basis with O(1)
conditioning; the constant column folds into a per-o bias added during PSUM
evacuation (shipped x16; the host divides the gathered output by 16, exact in
fp32).

Sharding: data-parallel over batch b (16384 -> 2048/core); coeffs replicated.
x is pre-transposed on the host so the contraction dim (c_in) lands on SBUF
partitions; the kernel computes y^T per core and the host transposes back.

Everything on device is fp16: x ships as fp16 (2MB/core), weights as fp16
scaled x16 on the host (4MB/core; the raw values ~1e-4 would graze the fp16
subnormal floor), and the whole basis chain computes in fp16 (10 mantissa
bits -> measured end-to-end error ~1.2e-3 vs the 2e-2 gate). fp16 matmuls
stream 1 cycle/row -- measured 213ns per 512-row matmul, the PE floor --
PSUM accumulates fp32, and the fp16-native chain needs no cast ops: ACT runs
the critical path (tanh -> tanh -> Square -> affine, 4 ops/row-block), DVE
runs the 9 off-critical multiply/affine ops at 16-bit double rate.

Batch schedule: 2048 rows/core = two interleaved 256-wide PSUM phases (8
banks, processed row-block by row-block so each 1MB weight block serves both
phases before the next is needed -- a single narrow phase demands weights at
300GB/s exactly during the DMA ramp and stalls LDWEIGHTS) followed by three
512-wide phases. The narrow head still buys the fast ramp-in: the 64KB first
x-sliver lands ~2.5us sooner than a 256KB one and the half-width tanh chain
halves the first-matmul latency.

Scheduling lessons baked in (from NTFF profiles of prior revisions): the
engine queues are strictly in-order, so EMISSION order is the schedule.
(1) PSUM evacuations for a phase are emitted after the NEXT phase's first
row-block: any earlier they head-of-line block the ACT/DVE queues waiting
for the accumulation group to stop (~2-3us PE stall per phase boundary,
which also drops the PE HAM clock gate 2.4->1.2GHz and bleeds another ~3us);
the banks are not reused until one more phase later (bufs=2), so late
evacuation is free. Evacs alternate ACT/DVE (bias via the per-partition AP
scalar of tensor_scalar). (2) All input DMAs ride the sync HWDGE ring in
exact consumption order -- the 16 DMA engines round-robin ACTIVE transfers,
so issuing a late-needed 1MB load early steals bandwidth from the critical
first tiles (measured: gpsimd SWDGE delivers only ~110GB/s per queue and
serialized the row-block-0 weight chunks, starving LDWEIGHTS ~10us).
(3) gpsimd (Q7) gets no elementwise work (measured ~1.5us per [128,512]
cast, 2-3x the cost model) and no PSUM access; it only issues the hidden
mid-phase output DMAs. The final phase's outputs use the two fast HWDGE
rings (sync/scalar, ~360GB/s). (4) A ~3.4us burst of fp32 dummy matmuls
warms the PE HAM clock gate during the DMA ramp; a tiny bias load goes first
on the gpsimd ring to absorb the ~1.5us DMA-engine doorbell-to-packet wake.
(5) tile_wait_until stamps per row-block bias the Tile scheduler toward
consumption order (its DMA-completion predictions are optimistic).
"""

import sys

if "/opt/trn_rl_repo" not in sys.path:
    sys.path.append("/opt/trn_rl_repo")

import numpy as np

import concourse.bacc as bacc
import concourse.tile as tile
from concourse import mybir
from concourse.bass_utils import run_bass_kernel_spmd

DEGREE = 8
B, C_IN, C_OUT = 16384, 512, 512
N_CORES = 8
NB = B // N_CORES            # 2048 batch rows per core
N_IB = C_IN // 128           # 4 contraction row-blocks
N_J = DEGREE                 # basis funcs phi_1..phi_8 (constant -> bias)
F32 = mybir.dt.float32
F16 = mybir.dt.float16
W_SCALE = 16.0               # host premultiplies weights+bias; host divides y

PAIR = [(0, 256), (256, 256)]                       # interleaved narrow phases
WIDE = [(512, 512), (1024, 512), (1536, 512)]
assert sum(w for _, w in PAIR + WIDE) == NB

_CACHE = {}


def _build():
    nc = bacc.Bacc("TRN2", target_bir_lowering=False, debug=False)
    # x ships pre-packed on the host into consumption order: one contiguous
    # [128, width] segment per (phase, row-block) so every x load is a single
    # DMA with fat (3.5-4KB) descriptors -- 256-column slices of the natural
    # [c_in, b] layout produce 512B descriptors and the early DMA window
    # measured only ~175GB/s, landing weights late.
    xt = nc.dram_tensor("xt", [128, NB * N_IB], F16, kind="ExternalInput")
    wmat = nc.dram_tensor("wmat", [C_IN, N_J * C_OUT], F16, kind="ExternalInput")
    biasv = nc.dram_tensor("biasv", [128, 4], F32, kind="ExternalInput")
    yt = nc.dram_tensor("yt", [C_OUT, NB], F32, kind="ExternalOutput")

    Tanh = mybir.ActivationFunctionType.Tanh
    Square = mybir.ActivationFunctionType.Square
    Identity = mybir.ActivationFunctionType.Identity
    ALU_MULT = mybir.AluOpType.mult
    ALU_ADD = mybir.AluOpType.add

    with tile.TileContext(nc) as tc:
        with (
            tc.tile_pool(name="const", bufs=1) as const_pool,
            tc.tile_pool(name="wts", bufs=1) as wpool,
            tc.tile_pool(name="pows", bufs=2) as ppool,
            tc.tile_pool(name="outs", bufs=2) as opool,
            tc.tile_pool(name="psum", bufs=2, space="PSUM") as pspool,
        ):
            # PE warm-up fodder while the first DMAs are in flight.
            dummy = const_pool.tile([128, 128], F32, tag="dummy")
            nc.gpsimd.memset(dummy[:], 0.0)
            cm1 = const_pool.tile([128, 1], F16, tag="cm1")
            nc.gpsimd.memset(cm1[:], -1.0)
            dps = pspool.tile([128, 512], F32, tag="ps3", name="dps")
            for _ in range(8):
                nc.tensor.matmul(
                    dps[:, 0:128], lhsT=dummy[:], rhs=dummy[:],
                    start=True, stop=True,
                )

            # Tiny bias load first on gpsimd: absorbs the DMA-engine wake.
            bias_t = const_pool.tile([128, 4], F32)
            nc.gpsimd.dma_start(out=bias_t[:], in_=biasv.ap())

            # All input loads on the sync HWDGE ring in consumption order.
            w_sb = [
                wpool.tile([128, N_J * C_OUT], F16, tag=f"wc{ib}", name=f"wc{ib}")
                for ib in range(N_IB)
            ]

            def load_w(ib, j0, j1):
                nc.sync.dma_start(
                    out=w_sb[ib][:, j0 * C_OUT : j1 * C_OUT],
                    in_=wmat.ap()[ib * 128 : (ib + 1) * 128, j0 * C_OUT : j1 * C_OUT],
                )

            # SBUF x tiles mirror the packed DRAM layout: the narrow pair in
            # one [128, 2048] tile (8 x 256-col segments in block order),
            # each wide phase in its own [128, 2048] tile.
            xpair_t = ppool.tile([128, 2048], F16, tag="xpair", bufs=1)
            xw_t = [
                ppool.tile([128, 2048], F16, tag=f"xw{pi}", bufs=1, name=f"xw{pi}")
                for pi in range(len(WIDE))
            ]
            # 64KB (ph0, ib0) sliver gates the tanh chain -> first matmul
            nc.sync.dma_start(out=xpair_t[:, 0:256], in_=xt.ap()[:, 0:256])
            load_w(0, 0, 1)
            load_w(0, 1, 2)
            load_w(0, 2, 3)
            load_w(0, 3, 4)
            nc.sync.dma_start(out=xpair_t[:, 256:2048], in_=xt.ap()[:, 256:2048])
            load_w(0, 4, 5)
            load_w(0, 5, 6)
            load_w(0, 6, 7)
            load_w(0, 7, 8)
            load_w(1, 0, 8)
            load_w(2, 0, 8)
            nc.sync.dma_start(out=xw_t[0][:], in_=xt.ap()[:, 2048:4096])
            load_w(3, 0, 8)
            nc.sync.dma_start(out=xw_t[1][:], in_=xt.ap()[:, 4096:6144])
            nc.sync.dma_start(out=xw_t[2][:], in_=xt.ap()[:, 6144:8192])

            def w_chunk(ib, j, oc):
                return w_sb[ib][:, j * C_OUT + oc * 128 : j * C_OUT + (oc + 1) * 128]

            def emit_chain_and_mms(ps, xsl, ib, wd, start_ib, stop_ib):
                # fp16 Chebyshev-product basis chain. Critical path
                # (tanh -> tanh -> Square -> affine) on ACT; DVE runs the
                # off-critical ops at 16-bit 2x rate.
                s = slice(0, wd)
                t1 = ppool.tile([128, 512], F16, tag="t1", bufs=4)
                f2 = ppool.tile([128, 512], F16, tag="f2", bufs=4)
                t2 = ppool.tile([128, 512], F16, tag="t2", bufs=4)
                u3 = ppool.tile([128, 512], F16, tag="u3", bufs=4)
                t3 = ppool.tile([128, 512], F16, tag="t3", bufs=4)
                f3 = ppool.tile([128, 512], F16, tag="f3", bufs=4)
                f4 = ppool.tile([128, 512], F16, tag="f4", bufs=4)
                t4 = ppool.tile([128, 512], F16, tag="t4", bufs=4)
                f5 = ppool.tile([128, 512], F16, tag="f5", bufs=4)
                f6 = ppool.tile([128, 512], F16, tag="f6", bufs=4)
                f7 = ppool.tile([128, 512], F16, tag="f7", bufs=4)
                f8 = ppool.tile([128, 512], F16, tag="f8", bufs=4)
                nc.scalar.activation(xsl, xsl, Tanh)
                nc.scalar.activation(t1[:, s], xsl, Tanh)
                nc.scalar.activation(f2[:, s], t1[:, s], Square)
                nc.scalar.activation(
                    t2[:, s], f2[:, s], Identity, bias=cm1[:], scale=2.0
                )
                nc.vector.tensor_scalar(
                    u3[:, s], f2[:, s], 4.0, -3.0, ALU_MULT, ALU_ADD
                )
                nc.vector.tensor_mul(t3[:, s], t1[:, s], u3[:, s])
                nc.vector.tensor_mul(f3[:, s], t1[:, s], t2[:, s])
                nc.vector.tensor_mul(f4[:, s], t2[:, s], t2[:, s])
                nc.vector.tensor_scalar(
                    t4[:, s], f4[:, s], 2.0, -1.0, ALU_MULT, ALU_ADD
                )
                nc.vector.tensor_mul(f5[:, s], t2[:, s], t3[:, s])
                nc.vector.tensor_mul(f6[:, s], t3[:, s], t3[:, s])
                nc.vector.tensor_mul(f7[:, s], t3[:, s], t4[:, s])
                nc.vector.tensor_mul(f8[:, s], t4[:, s], t4[:, s])
                chunks = [t1, f2, f3, f4, f5, f6, f7, f8]
                if ib < N_IB - 1:
                    order = [(j, oc) for j in range(N_J) for oc in range(4)]
                else:
                    # oc-major on the last row-block: groups finish staggered
                    order = [(j, oc) for oc in range(4) for j in range(N_J)]
                for j, oc in order:
                    nc.tensor.matmul(
                        ps[oc][:, s],
                        lhsT=w_chunk(ib, j, oc),
                        rhs=chunks[j][:, s],
                        start=(ib == start_ib and j == 0),
                        stop=(ib == stop_ib and j == N_J - 1),
                    )

            def emit_evacs(ps, off, wd, final):
                # Evacuate a completed phase's PSUM banks; called after the
                # NEXT phase's first row-block so the in-order ACT/DVE
                # queues never wait on a still-accumulating group.
                s = slice(0, wd)
                for oc in range(4):
                    osb = opool.tile(
                        [128, 512], F32, tag=f"osb{oc}", name=f"osb{oc}"
                    )
                    if oc % 2 == 0:
                        nc.scalar.activation(
                            osb[:, s], ps[oc][:, s], Identity,
                            bias=bias_t[:, oc : oc + 1],
                        )
                    else:
                        nc.vector.tensor_scalar(
                            osb[:, s], ps[oc][:, s], bias_t[:, oc : oc + 1],
                            None, ALU_ADD,
                        )
                    if final:
                        out_eng = (nc.scalar, nc.sync, nc.scalar, nc.sync)[oc]
                    else:
                        out_eng = nc.gpsimd
                    out_eng.dma_start(
                        out=yt.ap()[oc * 128 : (oc + 1) * 128, off : off + wd],
                        in_=osb[:, s],
                    )

            # Virtual-time stamps (ms): when each row-block's matmuls can
            # start (10.5us prologue + PE time so far).
            t_ms = 0.0105

            # --- interleaved narrow pair ---
            ps_pair = [
                [
                    pspool.tile(
                        [128, 512], F32, tag=f"ps{oc}", name=f"ps{oc}_p{phi}"
                    )
                    for oc in range(4)
                ]
                for phi in range(2)
            ]
            for ib in range(N_IB):
                for phi, (off, wd) in enumerate(PAIR):
                    seg = (ib * 2 + phi) * 256
                    with tc.tile_wait_until(t_ms):
                        emit_chain_and_mms(
                            ps_pair[phi], xpair_t[:, seg : seg + 256],
                            ib, wd, 0, N_IB - 1,
                        )
                    t_ms += wd * 128 * 0.4167 * 1e-6
            pending = [(ps_pair[0], PAIR[0][0], 256), (ps_pair[1], PAIR[1][0], 256)]

            # --- wide phases ---
            for pi, (off, wd) in enumerate(WIDE):
                ps = [
                    pspool.tile([128, 512], F32, tag=f"ps{oc}", name=f"ps{oc}_w{pi}")
                    for oc in range(4)
                ]
                final = pi == len(WIDE) - 1
                for ib in range(N_IB):
                    with tc.tile_wait_until(t_ms):
                        emit_chain_and_mms(
                            ps, xw_t[pi][:, ib * 512 : (ib + 1) * 512],
                            ib, wd, 0, N_IB - 1,
                        )
                        if ib == 0:
                            for pps, poff, pwd in pending:
                                emit_evacs(pps, poff, pwd, False)
                            pending = []
                    t_ms += wd * 128 * 0.4167 * 1e-6
                if final:
                    with tc.tile_wait_until(t_ms):
                        emit_evacs(ps, off, wd, True)
                else:
                    pending = [(ps, off, wd)]
    nc.compile()
    return nc


def _host_transform(cheby_coeffs):
    # Map Chebyshev coefficients onto the device phi basis:
    # phi = [T1, T1^2, T1*T2, T2^2, T2*T3, T3^2, T3*T4, T4^2] and a constant.
    # T_{2k} = 2*T_k^2 - 1, T_{m+n} = 2*T_m*T_n - T_{m-n} =>
    #   y = bias + (C1-C3-C5-C7)*T1 + sum_{d=2..8} 2*C_d * phi_{d-1}
    #   bias_o = sum_i (C0 - C2 - C4 - C6 - C8)
    C64 = cheby_coeffs.astype(np.float64)
    bias = (C64[..., 0] - C64[..., 2] - C64[..., 4] - C64[..., 6] - C64[..., 8]).sum(
        axis=0
    )
    W = np.empty((C_IN, C_OUT, N_J), np.float64)
    W[..., 0] = C64[..., 1] - C64[..., 3] - C64[..., 5] - C64[..., 7]
    for d in range(2, DEGREE + 1):
        W[..., d - 1] = 2.0 * C64[..., d]
    # [i, j*512+o]: per-partition-contiguous coefficient rows; fp16 on device,
    # premultiplied by W_SCALE (undone on the host) to clear the fp16
    # subnormal floor. The bias is folded in at the same scale.
    Wd = np.ascontiguousarray(
        (W.transpose(0, 2, 1).reshape(C_IN, N_J * C_OUT) * W_SCALE).astype(np.float16)
    )
    bias_dev = np.ascontiguousarray(
        (bias * W_SCALE).reshape(4, 128).T.astype(np.float32)
    )
    return Wd, bias_dev


def _pack_x(xTc):
    # Repack one core's [c_in, nb] slice into the device's consumption-ordered
    # [128, 8192] layout: 256-col segments for the interleaved narrow pair in
    # block order, then 512-col segments per wide-phase row-block.
    xq = xTc.astype(np.float16)
    out = np.empty((128, NB * N_IB), np.float16)
    col = 0
    for ib in range(N_IB):
        rows = xq[ib * 128 : (ib + 1) * 128]
        for phi, (off, wd) in enumerate(PAIR):
            out[:, col : col + wd] = rows[:, off : off + wd]
            col += wd
    for off, wd in WIDE:
        for ib in range(N_IB):
            out[:, col : col + wd] = xq[ib * 128 : (ib + 1) * 128, off : off + wd]
            col += wd
    assert col == NB * N_IB
    return np.ascontiguousarray(out)


def make_in_maps(x, cheby_coeffs):
    Wd, bias_dev = _host_transform(cheby_coeffs)
    xT = np.ascontiguousarray(x.T)                       # [c_in, b]
    return [
        {
            "xt": _pack_x(xT[:, c * NB : (c + 1) * NB]),
            "wmat": Wd,
            "biasv": bias_dev,
        }
        for c in range(N_CORES)
    ]


def kernel(x, cheby_coeffs):
    x = np.asarray(x, dtype=np.float32)
    cheby_coeffs = np.asarray(cheby_coeffs, dtype=np.float32)
    if "nc" not in _CACHE:
        _CACHE["nc"] = _build()
    nc = _CACHE["nc"]

    in_maps = make_in_maps(x, cheby_coeffs)
    res = run_bass_kernel_spmd(nc, in_maps, core_ids=list(range(N_CORES)))
    y = np.concatenate([res.results[c]["yt"].T for c in range(N_CORES)], axis=0)
    return (y * np.float32(1.0 / W_SCALE)).astype(np.float32)
